# revision 38
# baseline (speedup 1.0000x reference)
"""Trainium2 Bass kernel for nn_Attention_1056561955116 (sparse chunk attention).

Contract: kernel(**inputs) takes FULL unsharded numpy inputs (as produced by
the problem's setup_inputs) and returns the FULL [2, 2048, 1024] f32 output.

Sharding: 8 NeuronCores = 2 batches x 4 head-groups (4 heads / 256 inner dims
each). Per core: QKV projections (bf16 matmuls, f32 accum), qk-RMS-norm,
rotary, block-sparse attention (41 of 256 chunk pairs), output projection to a
partial [2048, 1024]; ReduceScatter(add) over the 4 cores of each batch; the
host concatenates the scattered row blocks.
"""

import os
import sys

import numpy as np

for _p in ("/opt/trn_rl_repo", "/root/.axon_site/_ro/trn_rl_repo"):
    if os.path.isdir(_p) and _p not in sys.path:
        sys.path.append(_p)

import concourse.bass as bass  # noqa: E402,F401
import concourse.mybir as mybir  # noqa: E402
import concourse.tile as tile  # noqa: E402
from concourse import bacc  # noqa: E402
from concourse.bass_utils import run_bass_kernel_spmd  # noqa: E402

try:
    import ml_dtypes

    BF16_NP = ml_dtypes.bfloat16
except ImportError:  # pragma: no cover
    BF16_NP = np.float32

F32 = mybir.dt.float32
BF16 = mybir.dt.bfloat16
AF = mybir.ActivationFunctionType
OP = mybir.AluOpType

B = 2
S = 2048
D = 1024
HEADS = 16
HD = 64
CHUNK = 128
N_CORES = 8
HG = 4          # heads per core
E = HG * HD     # 256 inner dims per core
NCH = S // CHUNK  # 16 chunks
GROUPS = [[0, 1, 2, 3], [4, 5, 6, 7]]
EPS = 1e-6


def _build_chunk_lists():
    n_cache = S // (2 * CHUNK)  # 8
    max_lookback = 5
    K = []  # K[j] = key chunks visible to query chunk j
    for j in range(NCH):
        ks = []
        rel = j - n_cache
        if rel >= 0:
            ks = [c for c in range(n_cache)
                  if c < rel and c >= rel - max_lookback]
        ks.append(j)
        K.append(ks)
    J = [[j for j in range(NCH) if c in K[j]] for c in range(NCH)]
    return K, J


K_J, J_C = _build_chunk_lists()
START_C = [K_J[j][0] for j in range(NCH)]
assert all(K_J[j][-1] == j for j in range(NCH))

# exp-batch groups per query bank-block jb: pack consecutive j's while the
# packed logits stay within 12 key chunks (1536 cols = 3 psum banks)
J_GROUPS = []
for jb in range(4):
    groups = []
    cur = []
    for j in range(4 * jb, 4 * jb + 4):
        if cur and sum(len(K_J[x]) for x in cur) + len(K_J[j]) > 12:
            groups.append(cur)
            cur = []
        cur.append(j)
    groups.append(cur)
    J_GROUPS.append(groups)


def _group_plan(grp):
    """Batched matmul plan for a query-chunk group: list of
    (c, j_first, j_count, col, start, stop) with uniform start/stop flags."""
    plan = []
    col = 0
    cs = sorted({c for j in grp for c in K_J[j]})
    for c in cs:
        js = [j for j in grp if c in K_J[j]]
        assert js == list(range(js[0], js[0] + len(js)))
        run = []
        for j in js:
            fl = (c == START_C[j], c == j)
            if run and run[-1][1] == fl:
                run[-1][0].append(j)
            else:
                run.append(([j], fl))
        for jlist, (st, sp_) in run:
            # split so each matmul's lt output stays within one psum bank
            rem = jlist
            while rem:
                room = (512 - col % 512) // 128
                take = min(len(rem), room) if room else min(len(rem), 4)
                plan.append((c, rem[0], take, col, st, sp_))
                col += 128 * take
                rem = rem[take:]
    return plan, col


GROUP_PLANS = {tuple(grp): _group_plan(grp)
               for jbg in J_GROUPS for grp in jbg}


def _dev2orig():
    # rotation pair i -> device lanes (32*(i//16) + i%16, +16): the rope
    # swap partner is lane ^ 16 inside each 32-lane group.
    d2o = np.zeros(HD, dtype=np.int64)
    for a in range(HD):
        q32, r0 = divmod(a, 32)
        o, r = divmod(r0, 16)
        d2o[a] = 2 * (16 * q32 + r) + o
    return d2o


DEV2ORIG = _dev2orig()
SWAP = DEV2ORIG[np.arange(HD) ^ 16]

_PROGRAM_CACHE = {}


class _Bacc(bacc.Bacc):
    def insert_act_table_loads(self):
        from concourse.hw_specs import get_activation_tables
        from concourse import bass_primitives_rust as _br

        has_activation = any(
            isinstance(i, mybir.InstActivation)
            for b in self.main_func.blocks
            for i in b.instructions
        )
        if not has_activation:
            return
        A = mybir.ActivationFunctionType
        keep_only_in = "natural_log_exp_and_others"
        steer = {A.Exp, A.Ln}
        tables = []
        for name, fns in get_activation_tables(self.m.arch).items():
            if name != keep_only_in:
                fns = {f for f in fns if f not in steer}
            tables.append((name, fns))
        import bass_rust as _bass_rust
        _bass_rust.insert_act_table_loads(self, tables)


def build_program(fold_w: bool):
    if fold_w in _PROGRAM_CACHE:
        return _PROGRAM_CACHE[fold_w]

    nc = _Bacc("TRN2", target_bir_lowering=False, debug=False,
               num_devices=N_CORES)

    xT = nc.dram_tensor("xT", [D, S], BF16, kind="ExternalInput")
    wqT = nc.dram_tensor("wqT", [D, E], BF16, kind="ExternalInput")
    wkT = nc.dram_tensor("wkT", [D, E], BF16, kind="ExternalInput")
    wvT = nc.dram_tensor("wvT", [D, E], BF16, kind="ExternalInput")
    woT = nc.dram_tensor("woT", [E, D], BF16, kind="ExternalInput")
    bqd = nc.dram_tensor("bq", [E], F32, kind="ExternalInput")
    bkd = nc.dram_tensor("bk", [E], F32, kind="ExternalInput")
    bod = nc.dram_tensor("bo_eff", [128, D], BF16, kind="ExternalInput")
    cosargd = nc.dram_tensor("cosarg", [128, S], F32, kind="ExternalInput")
    sinargd = nc.dram_tensor("sinarg", [128, S], F32, kind="ExternalInput")
    if fold_w:
        qwdevd = nc.dram_tensor("qw_dev", [128, 1], F32, kind="ExternalInput")
        qwswpd = nc.dram_tensor("qw_swp", [128, 1], F32, kind="ExternalInput")
        kwdevd = nc.dram_tensor("kw_dev", [128, 1], F32, kind="ExternalInput")
        kwswpd = nc.dram_tensor("kw_swp", [128, 1], F32, kind="ExternalInput")
    indd = nc.dram_tensor("ind", [128, 2], BF16, kind="ExternalInput")
    ones64d = nc.dram_tensor("ones64", [1, 64], F32, kind="ExternalInput")
    seld = nc.dram_tensor("sel", [128, 512], BF16, kind="ExternalInput")

    out_ext = nc.dram_tensor("out", [512, D], BF16, kind="ExternalOutput")
    partial = [nc.dram_tensor(f"partial{jb}", [512, D], BF16) for jb in range(4)]
    rs_out = [nc.dram_tensor(f"rs_out{jb}", [128, D], BF16) for jb in range(4)]

    with tile.TileContext(nc) as tc:
        with tc.tile_pool(name="persist", bufs=1) as pp, \
             tc.tile_pool(name="tmp", bufs=1) as tp, \
             tc.tile_pool(name="stage", bufs=3) as sp:

            x_sb = pp.tile([128, 8, S], BF16, name="x_sb")
            wq_sb = pp.tile([128, 8, E], BF16, name="wq_sb")
            wk_sb = pp.tile([128, 8, E], BF16, name="wk_sb")
            wv_sb = pp.tile([128, 8, E], BF16, name="wv_sb")
            wo_sb = pp.tile([128, 2, D], BF16, name="wo_sb")
            bq_sb = pp.tile([128, 2], F32, name="bq_sb")
            bk_sb = pp.tile([128, 2], F32, name="bk_sb")
            bo_sb = pp.tile([128, D], BF16, name="bo_sb")
            ind_sb = pp.tile([128, 2], BF16, name="ind_sb")
            sel_sb = pp.tile([128, 512], BF16, name="sel_sb")
            on64_sb = pp.tile([1, 64], F32, name="on64_sb")
            eps_sb = pp.tile([128, 1], F32, name="eps_sb")

            qt_sb = [pp.tile([128, S], BF16, name=f"qt{t}") for t in range(2)]
            kt_sb = [pp.tile([128, S], BF16, name=f"kt{t}") for t in range(2)]
            qh_sb = [pp.tile([128, S], BF16, name=f"qh{t}") for t in range(2)]
            kh_sb = [pp.tile([128, S], BF16, name=f"kh{t}") for t in range(2)]
            v_sb = pp.tile([128, NCH, HG * (HD + 1)], BF16, name="v_sb")
            o_fin = [pp.tile([128, S], BF16, name=f"ofin{t}") for t in range(2)]
            rall_sb = pp.tile([128, S], F32, name="rall_sb")
            rallb_sb = pp.tile([128, S], BF16, name="rallb_sb")
            cosq = pp.tile([128, S], BF16, name="cosq")
            sinq = pp.tile([128, S], BF16, name="sinq")
            if fold_w:
                cosk = pp.tile([128, S], BF16, name="cosk")
                sink = pp.tile([128, S], BF16, name="sink")
                qw_sb = pp.tile([128, 1], F32, name="qw_sb")
                qs_sb = pp.tile([128, 1], F32, name="qs_sb")
                kw_sb = pp.tile([128, 1], F32, name="kw_sb")
                ks_sb = pp.tile([128, 1], F32, name="ks_sb")
            else:
                cosk, sink = cosq, sinq

            nc.sync.dma_start(wq_sb[:], wqT.ap().rearrange("(c p) e -> p c e", p=128))
            nc.sync.dma_start(wk_sb[:], wkT.ap().rearrange("(c p) e -> p c e", p=128))
            nc.sync.dma_start(wv_sb[:], wvT.ap().rearrange("(c p) e -> p c e", p=128))
            nc.sync.dma_start(wo_sb[:], woT.ap().rearrange("(c p) d -> p c d", p=128))
            nc.sync.dma_start(bq_sb[:], bqd.ap().rearrange("(t p) -> p t", p=128))
            nc.sync.dma_start(bk_sb[:], bkd.ap().rearrange("(t p) -> p t", p=128))
            nc.sync.dma_start(bo_sb[:], bod[:])
            nc.sync.dma_start(ind_sb[:], indd[:])
            nc.sync.dma_start(sel_sb[:], seld[:])
            nc.sync.dma_start(on64_sb[:], ones64d[:])
            if fold_w:
                nc.sync.dma_start(qw_sb[:], qwdevd[:])
                nc.sync.dma_start(qs_sb[:], qwswpd[:])
                nc.sync.dma_start(kw_sb[:], kwdevd[:])
                nc.sync.dma_start(ks_sb[:], kwswpd[:])
            nc.any.memset(eps_sb[:], EPS)
            for m in range(4):
                for dc in range(8):
                    nc.sync.dma_start(
                        x_sb[:, dc, 512 * m:512 * (m + 1)],
                        xT[128 * dc:128 * (dc + 1), 512 * m:512 * (m + 1)])

            # rope tables (args are pre-range-reduced on host; cos = sin(x+pi/2))
            for m in range(4):
                for argd, dstt in ((cosargd, cosq), (sinargd, sinq)):
                    ang = tp.tile([128, 512], F32, tag="ang")
                    nc.sync.dma_start(ang[:], argd[:, 512 * m:512 * (m + 1)])
                    nc.scalar.activation(dstt[:, 512 * m:512 * (m + 1)],
                                         ang[:], AF.Sin)
            if fold_w:
                nc.vector.tensor_scalar_mul(cosk[:], cosq[:], kw_sb[:, 0:1])
                nc.vector.tensor_scalar_mul(sink[:], sinq[:], ks_sb[:, 0:1])
                nc.vector.tensor_scalar_mul(cosq[:], cosq[:], qw_sb[:, 0:1])
                nc.vector.tensor_scalar_mul(sinq[:], sinq[:], qs_sb[:, 0:1])

            # ---------------- projections ----------------
            with tc.tile_pool(name="pjqk", bufs=3, space="PSUM") as pjqk:
                for (w_sb, b_sb, dst) in ((wq_sb, bq_sb, qt_sb),
                                          (wk_sb, bk_sb, kt_sb)):
                    for t in range(2):
                        for m in range(4):
                            ps = pjqk.tile([128, 512], F32, tag="pjqk")
                            for dc in range(8):
                                nc.tensor.matmul(
                                    ps[:], w_sb[:, dc, 128 * t:128 * (t + 1)],
                                    x_sb[:, dc, 512 * m:512 * (m + 1)],
                                    start=(dc == 0), stop=(dc == 7))
                            nc.vector.tensor_scalar(
                                dst[t][:, 512 * m:512 * (m + 1)], ps[:],
                                b_sb[:, t:t + 1], None, OP.add)

            # ---------------- rms-norm factors ----------------
            # ssq rows packed k-first so the transpose input starts at
            # partition 0: s = 2*(1-ik) + t  (k tiles -> 0,1; q tiles -> 2,3)
            with tc.tile_pool(name="pssq", bufs=1, space="PSUM") as pqs, \
                 tc.tile_pool(name="pjv", bufs=2, space="PSUM") as pjv, \
                 tc.tile_pool(name="prb", bufs=2, space="PSUM") as prbp:
                # ssq rows land at partitions 32*s + {0,1}; the rest of the
                # psum tile is zeroed so downstream full-width ops are finite.
                pssq = pqs.tile([128, S], F32, tag="pssq")
                nc.vector.memset(pssq[:], 0.0)
                for ik, src in ((0, qt_sb), (1, kt_sb)):
                    for t in range(2):
                        sq = tp.tile([128, S], BF16, tag="sqt")
                        nc.vector.tensor_tensor(sq[:], src[t][:], src[t][:], OP.mult)
                        s_ = 2 * (1 - ik) + t
                        for m in range(4):
                            nc.tensor.matmul(
                                pssq[32 * s_:32 * s_ + 2, 512 * m:512 * (m + 1)],
                                ind_sb[:], sq[:, 512 * m:512 * (m + 1)],
                                start=True, stop=True, tile_position=(0, 32 * s_))
                for m in range(4):
                    sl = slice(512 * m, 512 * (m + 1))
                    nc.scalar.activation(rall_sb[:, sl], pssq[:, sl], AF.Ln,
                                         scale=1.0 / HD, bias=eps_sb[:, 0:1])
                    nc.scalar.activation(rall_sb[:, sl], rall_sb[:, sl],
                                         AF.Exp, scale=-0.5)
                    nc.vector.tensor_copy(rallb_sb[:, sl], rall_sb[:, sl])
                for tn in range(NCH):
                    ps = pjv.tile([128, E], F32, tag="pjv")
                    for dc in range(8):
                        nc.tensor.matmul(
                            ps[:], x_sb[:, dc, 128 * tn:128 * (tn + 1)],
                            wv_sb[:, dc, :], start=(dc == 0), stop=(dc == 7))
                    nc.vector.tensor_copy(
                        v_sb[:, tn].rearrange("p (h x) -> p h x", x=HD + 1)[:, :, :HD],
                        ps[:].rearrange("p (h d) -> p h d", d=HD))
                nc.any.memset(
                    v_sb[:].rearrange("p t (h x) -> p t h x", x=HD + 1)[:, :, :, HD:],
                    1.0)


                # rope + per-(head,token) scale application
                for i, (src, dst, ct, st) in (
                        (0, (qt_sb[0], qh_sb[0], cosq, sinq)),
                        (2, (kt_sb[0], kh_sb[0], cosk, sink)),
                        (1, (qt_sb[1], qh_sb[1], cosq, sinq)),
                        (3, (kt_sb[1], kh_sb[1], cosk, sink))):
                    sh = tp.tile([128, S], BF16, tag="sh")
                    nc.vector.stream_shuffle(sh[:], src[:],
                                             [l ^ 16 for l in range(32)])
                    t1 = tp.tile([128, S], BF16, tag="t1")
                    nc.vector.tensor_tensor(t1[:], src[:], ct[:], OP.mult)
                    t2 = tp.tile([128, S], BF16, tag="t2")
                    nc.vector.tensor_tensor(t2[:], sh[:], st[:], OP.mult)
                    nc.vector.tensor_tensor(t1[:], t1[:], t2[:], OP.add)
                    for m in range(4):
                        rb = prbp.tile([128, 512], F32, tag="rb")
                        nc.tensor.matmul(
                            rb[:], sel_sb[:, 128 * i:128 * (i + 1)],
                            rallb_sb[:, 512 * m:512 * (m + 1)],
                            start=True, stop=True)
                        nc.vector.tensor_tensor(
                            dst[:, 512 * m:512 * (m + 1)],
                            t1[:, 512 * m:512 * (m + 1)], rb[:], OP.mult)

            # ---------------- attention / normalize / out-proj / RS ----------------
            # jb-outer so the output projection and the ReduceScatter chunk for
            # token block jb overlap with the attention of block jb+1.
            rs_done = []
            with tc.tile_pool(name="attps", bufs=2, space="PSUM") as aps:
                for jb in range(4):
                    for h in range(HG):
                        th, pb = h // 2, 64 * (h % 2)
                        b32 = 32 * (h % 2)
                        pv = aps.tile([HD + 1, 512], F32, tag="pv")
                        prev = None
                        for gidx, grp in enumerate([*J_GROUPS[jb], None]):
                            if grp is not None:
                                plan, ncol = GROUP_PLANS[tuple(grp)]
                                lt = aps.tile([128, 1536], F32, tag="lt")
                                for (c, j0, jn, col, st_, sp_) in plan:
                                    nc.tensor.matmul(
                                        lt[:, col:col + 128 * jn],
                                        kh_sb[th][pb:pb + 64,
                                                  128 * c:128 * (c + 1)],
                                        qh_sb[th][pb:pb + 64,
                                                  128 * j0:128 * (j0 + jn)],
                                        start=True, stop=True)
                                pch = sp.tile([128, 1536], BF16, tag="pch")
                                nc.scalar.activation(pch[:, :ncol], lt[:, :ncol],
                                                     AF.Exp,
                                                     scale=1.0 / float(HD) ** 0.5)
                            if prev is not None:
                                pplan, ppch, gi = prev
                                for pi, (c, j0, jn, col, st_, sp_) in \
                                        enumerate(pplan):
                                    vsl = v_sb[:, c].rearrange(
                                        "p (h x) -> p h x", x=HD + 1)[:, h, :]
                                    jj = j0 - 4 * jb
                                    nc.tensor.matmul(
                                        pv[:, 128 * jj:128 * (jj + jn)], vsl,
                                        ppch[:, col:col + 128 * jn],
                                        start=(gi == 0 and pi == 0),
                                        stop=(gi == len(J_GROUPS[jb]) - 1
                                              and pi == len(pplan) - 1))
                            prev = ((plan, pch, gidx)
                                    if grp is not None else None)
                        # normalize: o = pv[:64] * exp(-ln(den)) (bcast on GpSimd)
                        usl = sp.tile([1, 512], F32, tag="usl")
                        nc.scalar.activation(usl[:], pv[HD:HD + 1, :], AF.Ln)
                        lnb = aps.tile([HD, 512], F32, tag="lt")
                        nc.tensor.matmul(lnb[:], on64_sb[0:1, :], usl[:],
                                         start=True, stop=True)
                        rdb = sp.tile([HD, 512], BF16, tag="rdb")
                        nc.scalar.activation(rdb[:], lnb[:], AF.Exp, scale=-1.0)
                        nc.vector.tensor_tensor(
                            o_fin[th][pb:pb + 64, 512 * jb:512 * (jb + 1)],
                            pv[:HD, :], rdb[:], OP.mult)

                    # output projection for this token block + RS chunk
                    for tn in range(4 * jb, 4 * jb + 4):
                        for dh in range(2):
                            ps = aps.tile([128, 512], F32, tag="pv")
                            for ec in range(2):
                                nc.tensor.matmul(
                                    ps[:], o_fin[ec][:, 128 * tn:128 * (tn + 1)],
                                    wo_sb[:, ec, 512 * dh:512 * (dh + 1)],
                                    start=(ec == 0), stop=(ec == 1))
                            st = sp.tile([128, 512], BF16, tag="ost")
                            nc.any.tensor_add(st[:], ps[:],
                                              bo_sb[:, 512 * dh:512 * (dh + 1)])
                            nc.sync.dma_start(
                                partial[jb][128 * (tn - 4 * jb):
                                            128 * (tn - 4 * jb + 1),
                                        512 * dh:512 * (dh + 1)], st[:])
                    nc.gpsimd.collective_compute(
                        "ReduceScatter", OP.add, replica_groups=GROUPS,
                        ins=[partial[jb][:]], outs=[rs_out[jb][:]])
                    nc.gpsimd.dma_start(out_ext[128 * jb:128 * (jb + 1)],
                                        rs_out[jb][:])

    nc.compile()
    _PROGRAM_CACHE[fold_w] = nc
    return nc


def _wrap_pi(x):
    return ((np.asarray(x, np.float64) + np.pi) % (2 * np.pi) - np.pi).astype(
        np.float32)


def prep_core_inputs(x, mask, freqs, Wq, bq, Wk, bk, Wv, bv, Wo, bo, qw, kw):
    qw = np.asarray(qw, np.float32)
    kw = np.asarray(kw, np.float32)
    fold_w = not (np.all(qw == 1.0) and np.all(kw == 1.0))

    perm = (np.arange(HEADS)[:, None] * HD + DEV2ORIG[None, :]).reshape(-1)
    Wq_p = np.asarray(Wq, np.float32)[perm]
    Wk_p = np.asarray(Wk, np.float32)[perm]
    bq_p = np.asarray(bq, np.float32)[perm]
    bk_p = np.asarray(bk, np.float32)[perm]

    fr = np.asarray(freqs, np.float64)[:, DEV2ORIG].T  # [64, S] dev-lane order
    cos_a = _wrap_pi(fr + np.pi / 2)
    sign = np.where((np.arange(HD) % 32) < 16, -1.0, 1.0)
    sin_a = _wrap_pi(fr * sign[:, None])
    cosarg = np.concatenate([cos_a, cos_a], axis=0)
    sinarg = np.concatenate([sin_a, sin_a], axis=0)

    ind = np.zeros((128, 2), np.float32)
    ind[:64, 0] = 1.0
    ind[64:, 1] = 1.0
    # rall rows (within [128, S], at partitions 32*s + {0,1}):
    # s = 0,1 -> k tiles; s = 2,3 -> q tiles. sel broadcasts row -> tile:
    # tiles i = 0,1 (q): src row 64 + 32*i + p//64; i = 2,3 (k): 32*(i-2) + p//64
    sel = np.zeros((128, 512), np.float32)
    for i in range(4):
        base = 64 + 32 * i if i < 2 else 32 * (i - 2)
        for p in range(128):
            sel[base + p // 64, 128 * i + p] = 1.0


    x = np.asarray(x, np.float32)
    Wo = np.asarray(Wo, np.float32)
    bv = np.asarray(bv, np.float32)
    bo = np.asarray(bo, np.float32)
    Wv = np.asarray(Wv, np.float32)

    in_maps = []
    for core in range(N_CORES):
        b, g = divmod(core, 4)
        esl = slice(E * g, E * (g + 1))
        woT = np.ascontiguousarray(Wo[:, esl].T)     # [256, 1024]
        bo_eff = bv[esl] @ woT + (bo if g == 0 else 0.0)
        m = {
            "xT": np.ascontiguousarray(x[b].T).astype(BF16_NP),
            "wqT": np.ascontiguousarray(Wq_p[esl].T).astype(BF16_NP),
            "wkT": np.ascontiguousarray(Wk_p[esl].T).astype(BF16_NP),
            "wvT": np.ascontiguousarray(Wv[esl].T).astype(BF16_NP),
            "woT": woT.astype(BF16_NP),
            "bq": bq_p[esl].copy(),
            "bk": bk_p[esl].copy(),
            "bo_eff": np.broadcast_to(bo_eff.astype(BF16_NP),
                                      (128, D)).copy(),
            "cosarg": cosarg,
            "sinarg": sinarg,
            "ind": ind.astype(BF16_NP),
            "sel": sel.astype(BF16_NP),
                    "ones64": np.ones((1, 64), np.float32),
        }
        if fold_w:
            m["qw_dev"] = np.tile(qw[DEV2ORIG], 2).reshape(128, 1)
            m["qw_swp"] = np.tile(qw[SWAP], 2).reshape(128, 1)
            m["kw_dev"] = np.tile(kw[DEV2ORIG], 2).reshape(128, 1)
            m["kw_swp"] = np.tile(kw[SWAP], 2).reshape(128, 1)
        in_maps.append(m)
    return in_maps, fold_w


def run_cores(in_maps, fold_w, **kw):
    nc = build_program(fold_w)
    return run_bass_kernel_spmd(nc, in_maps, list(range(N_CORES)), **kw)


def kernel(**inputs):
    mask = np.asarray(inputs["mask"])
    assert mask.all(), "kernel specialized for all-true mask (spec fill=ones)"
    in_maps, fold_w = prep_core_inputs(**inputs)
    res = run_cores(in_maps, fold_w).results
    out = np.empty((B, S, D), np.float32)
    for core in range(N_CORES):
        b, g = divmod(core, 4)
        o = res[core]["out"].astype(np.float32)
        for jb in range(4):
            out[b, 512 * jb + 128 * g:512 * jb + 128 * (g + 1), :] = \
                o[128 * jb:128 * (jb + 1)]
    return out
